# revision 21
# baseline (speedup 1.0000x reference)
"""MoE all-to-all dispatcher kernel for one TRN2 chip (8 NeuronCores).

The reference dispatches tokens to experts (stable-sort by expert id,
gather), applies identity experts, then inverts the permutation and does
the top-k weighted combine.  Permute followed by its inverse is the
identity, so the dispatcher reduces to a per-token scale:

    out[t, :] = hidden[t, :] * (w[t, 0] + w[t, 1])

which is a pure memory-bound elementwise kernel.  Tokens are sharded
across the 8 cores; routing_indices never affect the output.

Raw bacc implementation (no TileContext): the Tile entry/exit barriers
cost ~15us on a ~94us-roofline kernel.  Pipeline:
  sync engine   : issues hidden-state load DMAs (HWDGE ring 0)
  vector engine : wsum = w0 + w1 once, then per-tile tensor_scalar mul
  scalar engine : issues output store DMAs (HWDGE ring 1), waits for
                  completion of all stores at the end
Each DMA gets a dedicated one-shot semaphore (wait >=16 = all 16 SDMA
engines of that exact transfer completed); all are cleared up front
behind a barrier so repeated NEFF executions start clean.
"""

import os

import numpy as np

from concourse import bacc, mybir
from concourse.bass_utils import run_bass_kernel_spmd

N_CORES = 8
T, H, TOPK = 32768, 1024, 2
T_SHARD = T // N_CORES          # 4096 tokens per core
P = 128                         # SBUF partitions
N_BLOCKS = T_SHARD // P         # 32 blocks of 128 tokens

BLK = int(os.environ.get("KBLK", "2"))     # blocks per mid-schedule tile
NSLOTS = int(os.environ.get("KSLOTS", "12"))
TAPER = int(os.environ.get("KTAPER", "0"))  # 1-block tiles at head/tail

_cached = {}


def _schedule():
    head = [1] * TAPER
    tail = [1] * TAPER
    mid = N_BLOCKS - len(head) - len(tail)
    assert mid % BLK == 0
    return head + [BLK] * (mid // BLK) + tail


def build_nc():
    nc = bacc.Bacc(None, target_bir_lowering=False)
    hs = nc.declare_dram_parameter(
        "hidden_states", [T_SHARD, H], mybir.dt.float32, isOutput=False)
    # host pre-permutes weights to [p, n, k] (token n*128+p) so this DMA is
    # one contiguous 32KB transfer instead of 4096 8-byte descriptors
    w = nc.declare_dram_parameter(
        "routing_weights", [P, N_BLOCKS, TOPK], mybir.dt.float32,
        isOutput=False)
    out = nc.declare_dram_parameter(
        "out", [T_SHARD, H], mybir.dt.float32, isOutput=True)

    sched = _schedule()
    n_seg = len(sched)
    offs = np.cumsum([0] + sched)  # block offset of each segment

    # One-shot semaphore per DMA.  A shared cumulative DMA sem is NOT sound
    # here: each dma_start's 16 per-SDMA-engine completions land
    # independently, so with several DMAs in flight a wait for 16*(k+1) can
    # be satisfied by later loads' fast engines while a slow engine (7/15
    # are documented stragglers) still owes load k's partition band.  With a
    # dedicated sem, >=16 requires all 16 engines of that exact DMA.
    ld_sems = [nc.alloc_semaphore(f"ld{k}") for k in range(n_seg)]
    st_sems = [nc.alloc_semaphore(f"st{k}") for k in range(n_seg)]
    w_sem = nc.alloc_semaphore("w_sem")
    v_sem = nc.alloc_semaphore("v_sem")
    all_sems = ld_sems + st_sems + [w_sem, v_sem]
    sem_nums = sorted(s.num for s in all_sems)
    assert sem_nums[-1] - sem_nums[0] == len(all_sems) - 1, sem_nums
    sem_range = range(sem_nums[0], sem_nums[-1] + 1)

    # Semaphores persist across NEFF executions: clear ours up front and
    # barrier so no engine races past a wait on a stale count.
    nc.gpsimd.dma_reset(sem_range)
    nc.gpsimd.sem_clear(sem_range)
    nc.all_engine_barrier()

    w_tile = nc.alloc_sbuf_tensor("w_tile", [P, N_BLOCKS, TOPK],
                                  mybir.dt.float32)
    wsum = nc.alloc_sbuf_tensor("wsum", [P, N_BLOCKS], mybir.dt.float32)
    in_slots = [
        nc.alloc_sbuf_tensor(f"in{s}", [P, BLK, H], mybir.dt.float32)
        for s in range(NSLOTS)
    ]
    out_slots = [
        nc.alloc_sbuf_tensor(f"o{s}", [P, BLK, H], mybir.dt.float32)
        for s in range(NSLOTS)
    ]

    def dram_ap(param, k):
        lo, blk = offs[k] * P, sched[k]
        return param[lo:lo + blk * P, :].rearrange("(b p) h -> p b h", p=P)

    # --- sync engine: hidden loads only (HWDGE ring 0) ---
    for k in range(n_seg):
        if k >= NSLOTS:
            # in-slot free once compute k-NSLOTS retired
            nc.sync.wait_ge(v_sem, k - NSLOTS + 1)
        nc.sync.dma_start(
            in_slots[k % NSLOTS][:, :sched[k], :], dram_ap(hs, k)
        ).then_inc(ld_sems[k], 16)

    # --- vector engine: wsum once, then scaled copies ---
    nc.vector.wait_ge(w_sem, 16)
    nc.vector.tensor_add(wsum[:], w_tile[:, :, 0], w_tile[:, :, 1])
    for k in range(n_seg):
        nc.vector.wait_ge(ld_sems[k], 16)
        if k >= NSLOTS:
            # out-slot free once store k-NSLOTS completed
            nc.vector.wait_ge(st_sems[k - NSLOTS], 16)
        ins = in_slots[k % NSLOTS]
        outs = out_slots[k % NSLOTS]
        last = None
        for b in range(sched[k]):
            col = offs[k] + b
            last = nc.vector.tensor_scalar_mul(
                outs[:, b, :], ins[:, b, :], wsum[:, col:col + 1])
        last.then_inc(v_sem, 1)

    # --- scalar engine: weight load first (ring 1, idle early), stores ---
    nc.scalar.dma_start(w_tile[:], w[:]).then_inc(w_sem, 16)
    for k in range(n_seg):
        nc.scalar.wait_ge(v_sem, k + 1)
        nc.scalar.dma_start(
            dram_ap(out, k), out_slots[k % NSLOTS][:, :sched[k], :]
        ).then_inc(st_sems[k], 16)
    for k in range(n_seg):
        nc.scalar.wait_ge(st_sems[k], 16)

    nc.compile()
    return nc


def run(hidden_states, routing_weights, trace=False):
    if "nc" not in _cached:
        _cached["nc"] = build_nc()
    nc = _cached["nc"]
    in_maps = [
        {
            "hidden_states": np.ascontiguousarray(
                hidden_states[c * T_SHARD:(c + 1) * T_SHARD]),
            "routing_weights": np.ascontiguousarray(
                routing_weights[c * T_SHARD:(c + 1) * T_SHARD]
                .reshape(N_BLOCKS, P, TOPK).transpose(1, 0, 2)),
        }
        for c in range(N_CORES)
    ]
    res = run_bass_kernel_spmd(nc, in_maps, core_ids=list(range(N_CORES)),
                               trace=trace)
    out = np.concatenate([res.results[c]["out"] for c in range(N_CORES)],
                         axis=0)
    return out, res


def kernel(hidden_states, routing_indices, routing_weights):
    hidden_states = np.asarray(hidden_states, dtype=np.float32)
    routing_weights = np.asarray(routing_weights, dtype=np.float32)
    out, _ = run(hidden_states, routing_weights, trace=False)
    return out
